# revision 12
# baseline (speedup 1.0000x reference)
"""Trainium2 Bass kernel for nn_CenterIdLoss (segment_reduce).

Math restructuring: the reference computes, with S = segment_sum(feat, label)
[C, C] and cnt = bincount(label):

    center[i] = S[label[i]] / cnt[label[i]]
    loss = mean_i( lse(center[i]) - center[i, label[i]] ) / (n / NUM_POS)

Every sample with the same label shares the same center row, so the per-sample
softmax collapses to a per-class expression:

    loss = (1/(n*m)) * sum_c [ cnt_c * log(ssum_c) - S[c, c] ]
      ssum_c = sum_j exp(S[c, j] / cnt_c)        (cnt clamped to >= 1)

No row-max subtraction is needed: |S[c,j]/cnt_c| is a mean of standard-normal
features, bounded by max|feat| (~6), so exp never overflows fp32.

Sharding: by label. Each core owns 512 classes, chosen by greedy bin-packing of
the label histogram so every core receives ~n/8 samples (cap = max bucket).
The host permutes rows of feat so each core gets exactly its classes' rows
(sorted by local class id), shipped as one fused [cap, 4098] array
([feat[i,label[i]], 1] extra columns + feat row), plus a tiny int32 local-label
vector. On device a one-hot block is built from the labels (iota + is_equal)
and the segment-sum becomes a sparse block one-hot matmul accumulated in PSUM
quarter-row phases; S[c,c] and counts fall out of the same matmul applied to
the two extra columns. No cross-core collectives; the host sums the 8 per-core
partial losses (the unshard step).
"""

import os
import numpy as np
from contextlib import ExitStack

N_TOTAL = 8192
C = 4096
NUM_POS = 4
NCORES = 8
CPC = C // NCORES  # classes per core = 512
P = 128
NM = CPC // P      # M-chunks per core = 4
NPH = 4            # PSUM phases per M-chunk (1024 feature cols each)
PHW = C // NPH     # 1024
NEX = 2            # extra cols: [feat[i, label[i]], 1]
FUSED = NEX + C    # 4098 columns: [diag, one] | feat
SCALE = 1.0 / (N_TOTAL * (N_TOTAL // NUM_POS))  # 2^-24

_compile_cache = {}


def _host_shard(feat, label):
    """Assign classes to cores by sample-count bin-packing, permute rows, and
    build the fused per-core inputs. Host work is index manipulation on
    `label` (plus row gathers)."""
    label = np.asarray(label).astype(np.int64)
    feat = np.asarray(feat)
    if feat.dtype != np.float32:
        feat = feat.astype(np.float32)
    counts = np.bincount(label, minlength=C)

    # Greedy LPT: biggest classes first onto the least-loaded core that still
    # has class slots. Gives per-core sample loads within ~1 of n/8.
    order_cls = np.argsort(-counts, kind="stable")
    load = np.zeros(NCORES, np.int64)
    slots = np.full(NCORES, CPC, np.int64)
    assign = np.empty(C, np.int64)
    for cls in order_cls:
        cands = np.nonzero(slots > 0)[0]
        tgt = cands[np.argmin(load[cands])]
        assign[cls] = tgt
        load[tgt] += counts[cls]
        slots[tgt] -= 1

    cap = int(load.max())
    cap = max(cap, P)
    nk = -(-cap // P)

    kset_lo = [10 ** 9] * NM
    kset_hi = [-1] * NM
    per_core = []
    for c in range(NCORES):
        cls_c = np.nonzero(assign == c)[0]
        # local index: spread classes (sorted by count desc) round-robin over
        # the NM M-chunks so each chunk gets ~equal sample mass
        cls_sorted = cls_c[np.argsort(-counts[cls_c], kind="stable")]
        local_of = np.empty(CPC, np.int64)
        ranks = np.arange(CPC)
        local_of[:] = (ranks % NM) * P + ranks // NM
        # map: global class -> local index
        lmap = np.full(C, -1, np.int64)
        lmap[cls_sorted] = local_of
        sel = np.nonzero(lmap[label] >= 0)[0]
        lab_loc = lmap[label[sel]]
        srt = np.argsort(lab_loc, kind="stable")
        idx = sel[srt]
        lab = lab_loc[srt]
        b = len(idx)
        if b:
            kk = np.arange(b) // P
            mm = lab // P
            for m in range(NM):
                s = mm == m
                if s.any():
                    kset_lo[m] = min(kset_lo[m], int(kk[s].min()))
                    kset_hi[m] = max(kset_hi[m], int(kk[s].max()))
        per_core.append((idx, lab, b))

    ksets = []
    for m in range(NM):
        if kset_hi[m] < 0:
            ksets.append([0])
        else:
            ksets.append(list(range(kset_lo[m], kset_hi[m] + 1)))

    in_maps = []
    for c in range(NCORES):
        idx, lab, b = per_core[c]
        fused = np.zeros((cap, FUSED), np.float32)
        labv = np.full(nk * P, -1, np.float32)
        if b:
            fused[:b, NEX:] = feat[idx]
            if b < cap:
                fused[b:, NEX:] = feat[idx[-1]]
            fused[:b, 0] = feat[idx, label[idx]]
            fused[:b, 1] = 1.0
            labv[:b] = lab.astype(np.float32)
        in_maps.append({"fused": fused, "labels": labv})
    return cap, tuple(tuple(s) for s in ksets), in_maps


def _build(cap, ksets, reps=1):
    """Build and compile the SPMD single-core program (same for all cores)."""
    import concourse.tile as tile
    import concourse.mybir as mybir
    from concourse import bacc

    f32 = mybir.dt.float32
    f32r = mybir.dt.float32r
    i32 = mybir.dt.int32
    nk = -(-cap // P)
    pk = [min(P, cap - P * k) for k in range(nk)]

    nc = bacc.Bacc("TRN2", target_bir_lowering=False, debug=False,
                   num_devices=NCORES)
    fused_d = nc.dram_tensor("fused", [cap, FUSED], f32r, kind="ExternalInput")
    lab_d = nc.dram_tensor("labels", [nk * P], f32, kind="ExternalInput")
    out_d = nc.dram_tensor("out", [1, 1], f32, kind="ExternalOutput")

    with tile.TileContext(nc) as tc, ExitStack() as ctx:
        fp = ctx.enter_context(tc.tile_pool(name="fusedp", bufs=nk + (1 if reps > 1 else 0)))
        ohp = ctx.enter_context(tc.tile_pool(name="ohp", bufs=10))
        sp = ctx.enter_context(tc.tile_pool(name="stat", bufs=3))
        lp = ctx.enter_context(tc.tile_pool(name="labp", bufs=2))
        scr = ctx.enter_context(tc.tile_pool(name="scr", bufs=3))
        ppx = ctx.enter_context(tc.tile_pool(name="psx", bufs=1, space="PSUM"))
        pph = ctx.enter_context(tc.tile_pool(name="psph", bufs=3, space="PSUM"))

        def one_pass():
            # labels laid out [P, nk]: element (p, k) = lab[k*P + p]
            lab_sb = lp.tile([P, nk], f32, tag="lab")
            nc.scalar.dma_start(lab_sb[:], lab_d[:].rearrange("(k p) -> p k", p=P))
            iota_t = lp.tile([P, P], f32, tag="iota")
            nc.gpsimd.iota(iota_t[:], pattern=[[1, P]], base=0, channel_multiplier=0,
                           allow_small_or_imprecise_dtypes=True)

            tiles = []
            for k in range(nk):
                t = fp.tile([pk[k], FUSED], f32r, tag="fused")
                nc.sync.dma_start(t[:], fused_d[P * k:P * k + pk[k], :])
                tiles.append(t)

            ext = ppx.tile([P, 8], f32, tag="ext")  # (d_m, cnt_m) pairs, 1 bank
            d_all = sp.tile([P, NM], f32, tag="dall")
            inv_all = sp.tile([P, NM], f32, tag="inv")
            cnt_all = sp.tile([P, NM], f32, tag="cnt")
            ssum_ph = sp.tile([P, NM * NPH], f32, tag="ssph")

            for m in range(NM):
                ks = ksets[m]
                # one-hot blocks for this m-chunk, built from labels
                ohs = {}
                for k in ks:
                    oh = ohp.tile([P, P], f32r, tag="oh")
                    # oh[p,f] = ((iota[f] - lab[p]) == -128m)
                    nc.vector.tensor_scalar(
                        oh[0:pk[k], :], iota_t[0:pk[k], :],
                        lab_sb[0:pk[k], k:k + 1],
                        float(-(P * m)),
                        op0=mybir.AluOpType.subtract,
                        op1=mybir.AluOpType.is_equal)
                    ohs[k] = oh
                # counts + diagonal for this m-chunk (sequential groups in the
                # shared extras bank)
                for j, k in enumerate(ks):
                    nc.tensor.matmul(
                        ext[:, 2 * m:2 * m + 2], ohs[k][0:pk[k], :],
                        tiles[k][:, 0:2],
                        start=(j == 0), stop=(j == len(ks) - 1))
                nc.vector.tensor_copy(cnt_all[:, m:m + 1], ext[:, 2 * m + 1:2 * m + 2])
                nc.vector.tensor_copy(d_all[:, m:m + 1], ext[:, 2 * m:2 * m + 1])
                cc = sp.tile([P, 1], f32, tag="cc")
                nc.vector.tensor_scalar_max(cc[:], ext[:, 2 * m + 1:2 * m + 2], 1.0)
                nc.vector.reciprocal(inv_all[:, m:m + 1], cc[:])

                for ph in range(NPH):
                    pt = pph.tile([P, PHW], f32, tag="ph")
                    for j, k in enumerate(ks):
                        for s in range(PHW // 512):
                            col = NEX + PHW * ph + 512 * s
                            nc.tensor.matmul(
                                pt[:, 512 * s:512 * (s + 1)], ohs[k][0:pk[k], :],
                                tiles[k][:, col:col + 512],
                                start=(j == 0), stop=(j == len(ks) - 1))
                    et = scr.tile([P, PHW], f32, tag="escr")
                    nc.scalar.activation(et[:], pt[:],
                                         mybir.ActivationFunctionType.Exp,
                                         scale=inv_all[:, m:m + 1],
                                         accum_out=ssum_ph[:, NPH * m + ph:NPH * m + ph + 1])

            # --- epilogue: lse terms for all 512 classes at once -------------
            ssum_all = sp.tile([P, NM], f32, tag="ssum")
            nc.vector.reduce_sum(ssum_all[:].rearrange("p (m one) -> p m one", one=1),
                                 ssum_ph[:].rearrange("p (m h) -> p m h", h=NPH),
                                 axis=mybir.AxisListType.X)
            ln_all = sp.tile([P, NM], f32, tag="ln")
            nc.scalar.activation(ln_all[:], ssum_all[:],
                                 mybir.ActivationFunctionType.Ln)
            x1 = sp.tile([P, NM], f32, tag="x1")
            nc.vector.tensor_mul(x1[:], cnt_all[:], ln_all[:])
            t_col = sp.tile([P, NM], f32, tag="tcol")
            nc.vector.tensor_sub(t_col[:], x1[:], d_all[:])
            tsum = sp.tile([P, 1], f32, tag="tsum")
            nc.vector.reduce_sum(tsum[:], t_col[:], axis=mybir.AxisListType.X)
            ones_t = sp.tile([P, 1], f32, tag="ones")
            nc.vector.memset(ones_t[:], 1.0)
            nc.tensor.matmul(ext[0:1, 0:1], tsum[:], ones_t[:],
                             start=True, stop=True)
            res = sp.tile([1, 1], f32, tag="res")
            nc.scalar.mul(res[:], ext[0:1, 0:1], SCALE)
            nc.scalar.dma_start(out_d[:, :], res[:])

        for _ in range(reps):
            one_pass()

    nc.compile()
    return nc


def _get_program(cap, ksets, reps=1):
    key = (cap, ksets, reps)
    if key not in _compile_cache:
        _compile_cache[key] = _build(cap, ksets, reps)
    return _compile_cache[key]


def kernel(**inputs):
    feat = inputs["feat"]
    label = inputs["label"]
    assert feat.shape == (N_TOTAL, C), feat.shape
    cap, ksets, in_maps = _host_shard(feat, label)
    nc = _get_program(cap, ksets)

    from concourse.bass_utils import run_bass_kernel_spmd
    res = run_bass_kernel_spmd(nc, in_maps, list(range(NCORES)))
    total = np.float32(0.0)
    for r in res.results:
        total += np.float32(r["out"].reshape(-1)[0])
    return np.asarray(total, dtype=np.float32)


# revision 13
# speedup vs baseline: 1.1254x; 1.1254x over previous
"""Trainium2 Bass kernel for nn_CenterIdLoss (segment_reduce).

Math restructuring: the reference computes, with S = segment_sum(feat, label)
[C, C] and cnt = bincount(label):

    center[i] = S[label[i]] / cnt[label[i]]
    loss = mean_i( lse(center[i]) - center[i, label[i]] ) / (n / NUM_POS)

Every sample with the same label shares the same center row, so the per-sample
softmax collapses to a per-class expression:

    loss = (1/(n*m)) * sum_c [ cnt_c * log(ssum_c) - S[c, c] ]
      ssum_c = sum_j exp(S[c, j] / cnt_c)        (cnt clamped to >= 1)

No row-max subtraction is needed: |S[c,j]/cnt_c| is a mean of standard-normal
features, bounded by max|feat| (~6), so exp never overflows fp32.

Sharding: by label. Each core owns 512 classes, chosen by greedy bin-packing of
the label histogram so every core receives ~n/8 samples (cap = max bucket).
The host permutes rows of feat so each core gets exactly its classes' rows
(sorted by local class id), shipped as one fused [cap, 4098] array
([feat[i,label[i]], 1] extra columns + feat row), plus a tiny int32 local-label
vector. On device a one-hot block is built from the labels (iota + is_equal)
and the segment-sum becomes a sparse block one-hot matmul accumulated in PSUM
quarter-row phases; S[c,c] and counts fall out of the same matmul applied to
the two extra columns. No cross-core collectives; the host sums the 8 per-core
partial losses (the unshard step).
"""

import os
import numpy as np
from contextlib import ExitStack

N_TOTAL = 8192
C = 4096
NUM_POS = 4
NCORES = 8
CPC = C // NCORES  # classes per core = 512
P = 128
NM = CPC // P      # M-chunks per core = 4
NPH = 4            # PSUM phases per M-chunk (1024 feature cols each)
PHW = C // NPH     # 1024
NEX = 2            # extra cols: [feat[i, label[i]], 1]
FUSED = NEX + C    # 4098 columns: [diag, one] | feat
SCALE = 1.0 / (N_TOTAL * (N_TOTAL // NUM_POS))  # 2^-24

_compile_cache = {}


def _host_shard(feat, label):
    """Assign classes to cores by sample-count bin-packing, permute rows, and
    build the fused per-core inputs. Host work is index manipulation on
    `label` (plus row gathers)."""
    label = np.asarray(label).astype(np.int64)
    feat = np.asarray(feat)
    if feat.dtype != np.float32:
        feat = feat.astype(np.float32)
    counts = np.bincount(label, minlength=C)

    # Greedy LPT: biggest classes first onto the least-loaded core that still
    # has class slots. Gives per-core sample loads within ~1 of n/8.
    order_cls = np.argsort(-counts, kind="stable")
    load = np.zeros(NCORES, np.int64)
    slots = np.full(NCORES, CPC, np.int64)
    assign = np.empty(C, np.int64)
    for cls in order_cls:
        cands = np.nonzero(slots > 0)[0]
        tgt = cands[np.argmin(load[cands])]
        assign[cls] = tgt
        load[tgt] += counts[cls]
        slots[tgt] -= 1

    cap = int(load.max())
    cap = max(cap, P)
    nk = -(-cap // P)

    kset_lo = [10 ** 9] * NM
    kset_hi = [-1] * NM
    per_core = []
    for c in range(NCORES):
        cls_c = np.nonzero(assign == c)[0]
        # local index: spread classes (sorted by count desc) round-robin over
        # the NM M-chunks so each chunk gets ~equal sample mass
        cls_sorted = cls_c[np.argsort(-counts[cls_c], kind="stable")]
        local_of = np.empty(CPC, np.int64)
        ranks = np.arange(CPC)
        local_of[:] = (ranks % NM) * P + ranks // NM
        # map: global class -> local index
        lmap = np.full(C, -1, np.int64)
        lmap[cls_sorted] = local_of
        sel = np.nonzero(lmap[label] >= 0)[0]
        lab_loc = lmap[label[sel]]
        srt = np.argsort(lab_loc, kind="stable")
        idx = sel[srt]
        lab = lab_loc[srt]
        b = len(idx)
        if b:
            kk = np.arange(b) // P
            mm = lab // P
            for m in range(NM):
                s = mm == m
                if s.any():
                    kset_lo[m] = min(kset_lo[m], int(kk[s].min()))
                    kset_hi[m] = max(kset_hi[m], int(kk[s].max()))
        per_core.append((idx, lab, b))

    ksets = []
    for m in range(NM):
        if kset_hi[m] < 0:
            ksets.append([0])
        else:
            ksets.append(list(range(kset_lo[m], kset_hi[m] + 1)))

    in_maps = []
    for c in range(NCORES):
        idx, lab, b = per_core[c]
        fused = np.zeros((cap, FUSED), np.float32)
        labv = np.full(nk * P, -1, np.float32)
        if b:
            fused[:b, NEX:] = feat[idx]
            if b < cap:
                fused[b:, NEX:] = feat[idx[-1]]
            fused[:b, 0] = feat[idx, label[idx]]
            fused[:b, 1] = 1.0
            labv[:b] = lab.astype(np.float32)
        in_maps.append({"fused": fused, "labels": labv})
    return cap, tuple(tuple(s) for s in ksets), in_maps


def _build(cap, ksets, reps=1):
    """Build and compile the SPMD single-core program (same for all cores)."""
    import concourse.tile as tile
    import concourse.mybir as mybir
    from concourse import bacc

    f32 = mybir.dt.float32
    f32r = mybir.dt.float32r
    i32 = mybir.dt.int32
    nk = -(-cap // P)
    pk = [min(P, cap - P * k) for k in range(nk)]

    nc = bacc.Bacc("TRN2", target_bir_lowering=False, debug=False,
                   num_devices=NCORES)
    fused_d = nc.dram_tensor("fused", [cap, FUSED], f32r, kind="ExternalInput")
    lab_d = nc.dram_tensor("labels", [nk * P], f32, kind="ExternalInput")
    out_d = nc.dram_tensor("out", [1, 1], f32, kind="ExternalOutput")

    with tile.TileContext(nc) as tc, ExitStack() as ctx:
        fp = ctx.enter_context(tc.tile_pool(name="fusedp", bufs=nk + (1 if reps > 1 else 0)))
        ohp = ctx.enter_context(tc.tile_pool(name="ohp", bufs=10))
        sp = ctx.enter_context(tc.tile_pool(name="stat", bufs=3))
        lp = ctx.enter_context(tc.tile_pool(name="labp", bufs=2))
        scr = ctx.enter_context(tc.tile_pool(name="scr", bufs=3))
        ppx = ctx.enter_context(tc.tile_pool(name="psx", bufs=1, space="PSUM"))
        pph = ctx.enter_context(tc.tile_pool(name="psph", bufs=3, space="PSUM"))

        def one_pass():
            # labels laid out [P, nk]: element (p, k) = lab[k*P + p]
            lab_sb = lp.tile([P, nk], f32, tag="lab")
            nc.sync.dma_start(lab_sb[:], lab_d[:].rearrange("(k p) -> p k", p=P))
            iota_t = lp.tile([P, P], f32, tag="iota")
            nc.gpsimd.iota(iota_t[:], pattern=[[1, P]], base=0, channel_multiplier=0,
                           allow_small_or_imprecise_dtypes=True)

            tiles = []
            for k in range(nk):
                t = fp.tile([pk[k], FUSED], f32r, tag="fused")
                nc.sync.dma_start(t[:], fused_d[P * k:P * k + pk[k], :])
                tiles.append(t)

            ext = ppx.tile([P, 8], f32, tag="ext")  # (d_m, cnt_m) pairs, 1 bank
            d_all = sp.tile([P, NM], f32, tag="dall")
            inv_all = sp.tile([P, NM], f32, tag="inv")
            cnt_all = sp.tile([P, NM], f32, tag="cnt")
            ssum_ph = sp.tile([P, NM * NPH], f32, tag="ssph")

            for m in range(NM):
                ks = ksets[m]
                # one-hot blocks for this m-chunk, built from labels
                ohs = {}
                for k in ks:
                    oh = ohp.tile([P, P], f32r, tag="oh")
                    # oh[p,f] = ((iota[f] - lab[p]) == -128m)
                    nc.vector.tensor_scalar(
                        oh[0:pk[k], :], iota_t[0:pk[k], :],
                        lab_sb[0:pk[k], k:k + 1],
                        float(-(P * m)),
                        op0=mybir.AluOpType.subtract,
                        op1=mybir.AluOpType.is_equal)
                    ohs[k] = oh
                # counts + diagonal for this m-chunk (sequential groups in the
                # shared extras bank)
                for j, k in enumerate(ks):
                    nc.tensor.matmul(
                        ext[:, 2 * m:2 * m + 2], ohs[k][0:pk[k], :],
                        tiles[k][:, 0:2],
                        start=(j == 0), stop=(j == len(ks) - 1))
                nc.vector.tensor_copy(cnt_all[:, m:m + 1], ext[:, 2 * m + 1:2 * m + 2])
                nc.vector.tensor_copy(d_all[:, m:m + 1], ext[:, 2 * m:2 * m + 1])
                cc = sp.tile([P, 1], f32, tag="cc")
                nc.vector.tensor_scalar_max(cc[:], ext[:, 2 * m + 1:2 * m + 2], 1.0)
                nc.vector.reciprocal(inv_all[:, m:m + 1], cc[:])

                for ph in range(NPH):
                    pt = pph.tile([P, PHW], f32, tag="ph")
                    for j, k in enumerate(ks):
                        for s in range(PHW // 512):
                            col = NEX + PHW * ph + 512 * s
                            nc.tensor.matmul(
                                pt[:, 512 * s:512 * (s + 1)], ohs[k][0:pk[k], :],
                                tiles[k][:, col:col + 512],
                                start=(j == 0), stop=(j == len(ks) - 1))
                    et = scr.tile([P, PHW], f32, tag="escr")
                    nc.scalar.activation(et[:], pt[:],
                                         mybir.ActivationFunctionType.Exp,
                                         scale=inv_all[:, m:m + 1],
                                         accum_out=ssum_ph[:, NPH * m + ph:NPH * m + ph + 1])

            # --- epilogue: lse terms for all 512 classes at once -------------
            ssum_all = sp.tile([P, NM], f32, tag="ssum")
            nc.vector.reduce_sum(ssum_all[:].rearrange("p (m one) -> p m one", one=1),
                                 ssum_ph[:].rearrange("p (m h) -> p m h", h=NPH),
                                 axis=mybir.AxisListType.X)
            ln_all = sp.tile([P, NM], f32, tag="ln")
            nc.scalar.activation(ln_all[:], ssum_all[:],
                                 mybir.ActivationFunctionType.Ln)
            x1 = sp.tile([P, NM], f32, tag="x1")
            nc.vector.tensor_mul(x1[:], cnt_all[:], ln_all[:])
            t_col = sp.tile([P, NM], f32, tag="tcol")
            nc.vector.tensor_sub(t_col[:], x1[:], d_all[:])
            tsum = sp.tile([P, 1], f32, tag="tsum")
            nc.vector.reduce_sum(tsum[:], t_col[:], axis=mybir.AxisListType.X)
            ones_t = sp.tile([P, 1], f32, tag="ones")
            nc.vector.memset(ones_t[:], 1.0)
            nc.tensor.matmul(ext[0:1, 0:1], tsum[:], ones_t[:],
                             start=True, stop=True)
            res = sp.tile([1, 1], f32, tag="res")
            nc.scalar.mul(res[:], ext[0:1, 0:1], SCALE)
            nc.sync.dma_start(out_d[:, :], res[:])

        for _ in range(reps):
            one_pass()

    nc.compile()
    return nc


def _get_program(cap, ksets, reps=1):
    key = (cap, ksets, reps)
    if key not in _compile_cache:
        _compile_cache[key] = _build(cap, ksets, reps)
    return _compile_cache[key]


def kernel(**inputs):
    feat = inputs["feat"]
    label = inputs["label"]
    assert feat.shape == (N_TOTAL, C), feat.shape
    cap, ksets, in_maps = _host_shard(feat, label)
    nc = _get_program(cap, ksets)

    from concourse.bass_utils import run_bass_kernel_spmd
    res = run_bass_kernel_spmd(nc, in_maps, list(range(NCORES)))
    total = np.float32(0.0)
    for r in res.results:
        total += np.float32(r["out"].reshape(-1)[0])
    return np.asarray(total, dtype=np.float32)
